# revision 28
# baseline (speedup 1.0000x reference)
"""Trainium2 Bass kernel for PointerAttention (Bahdanau additive attention).

    enc_t = encoder_outputs @ W1; dec_t = decoder_state @ W2
    log_score[b,d,e] = sum_k vt[k] * tanh(enc_t[b,e,k] + dec_t[b,d,k])
    returns (log_score + mask, log_score)

The 201M-element tanh tensor is never materialized: tanh(a+b) is
approximated by a separable bivariate polynomial in warped coordinates

    za = tanh(a/tau), zb = tanh(b/tau)
    tanh(a+b) ~= sum_{(p,q)} C_pq za^p zb^q     (full odd-degree grid)

so the (dec,enc) score reduces to matmuls over an expanded feature dim
(tensor engine at fp16 rate); elementwise work is only the warp
(scalar-engine passes) plus a shared power ladder on the vector engine.

Distribution: in this environment the dominant cost is host<->device
transfer (per-byte) plus per-call dispatch, not device compute, so the
kernel packs work onto few cores (weights would otherwise be replicated
per core) and loops over (batch, enc-block) tiles on-device.
"""

import os

import numpy as np

# Persistent XLA compilation cache: without it every run_bass_kernel_spmd
# call re-runs the full BIR->NEFF compile pipeline (~0.5s/call).
import jax

_CACHE_DIR = os.environ.get("BASS_JAX_CACHE", "/tmp/jax_bass_ptr_cache")
try:
    jax.config.update("jax_compilation_cache_dir", _CACHE_DIR)
    jax.config.update("jax_persistent_cache_min_compile_time_secs", 0.0)
    jax.config.update("jax_persistent_cache_min_entry_size_bytes", 0)
except Exception:
    pass

B, DEC, ENC, H = 4, 128, 512, 768
KCH = H // 128
HCH = H // 128

NCORES = 1  # tunable: 1/2/4/8; fewer cores = less replicated-weight traffic

TAU = 2.0
# bivariate polynomial terms: list of (p, q, coef)
TERMS = [(0, 1, 1.99033926), (0, 3, -1.79925282), (0, 5, 1.017906), (0, 9, -0.215433472), (1, 0, 1.99040857), (1, 2, -7.38985925), (1, 4, 10.2759259), (1, 6, -5.15726076), (2, 1, -7.3927193), (2, 3, 26.6806626), (2, 5, -28.1738826), (2, 9, 9.39193685), (3, 0, -1.82169664), (3, 2, 27.5479717), (3, 4, -72.3601525), (3, 6, 54.4204633), (3, 10, -3.66602355), (4, 1, 10.3621794), (4, 3, -68.2460749), (4, 5, 101.156957), (4, 9, -47.2775125), (5, 0, 1.06816096), (5, 2, -29.9933626), (5, 4, 108.180598), (5, 6, -97.5802979), (6, 1, -5.28888914), (6, 3, 48.3733341), (6, 5, -90.6168911), (6, 9, 54.631269), (7, 8, -35.905972), (7, 10, 74.0350356), (9, 0, -0.251279909), (9, 2, 10.6441498), (9, 4, -51.4730059), (9, 6, 81.6693111), (9, 10, -79.8753514), (10, 7, 18.6183337), (10, 9, -22.9504174), (11, 6, -27.2018259), (11, 8, 43.1152694)]

_COMPILED = {}


def _core_blocks(ncores):
    """Per-core work: list of (batch, enc_off, enc_len) blocks."""
    if ncores == 8:
        return [[(c // 2, (c % 2) * 256, 256)] for c in range(8)]
    if ncores == 4:
        return [[(c, 0, ENC)] for c in range(4)]
    if ncores == 2:
        return [[(2 * c, 0, ENC), (2 * c + 1, 0, ENC)] for c in range(2)]
    if ncores == 1:
        return [[(b, 0, ENC) for b in range(B)]]
    raise ValueError(ncores)


def _build_nc(ncores):
    import concourse.bacc as bacc
    import concourse.mybir as mybir
    import concourse.tile as tile

    fp16 = mybir.dt.float16
    fp32 = mybir.dt.float32
    AF = mybir.ActivationFunctionType

    terms_sorted = sorted(TERMS, key=lambda t: (max(t[0], t[1]), t[0]))
    pows = sorted(set([p for p, _, _ in TERMS] + [q for _, q, _ in TERMS]))

    blocks = _core_blocks(ncores)[0]  # every core has the same block shapes
    n_blk = len(blocks)
    batches = sorted(set(b for b, _, _ in blocks))
    n_bat = len(batches)
    enc_cols = sum(w for _, _, w in blocks)
    EW = blocks[0][2]  # enc width per block (uniform)

    nc = bacc.Bacc("TRN2", target_bir_lowering=False)

    # packed inputs: one fp16 blob [encT | decT | w1 | w2] (all H rows),
    # the mask as fp16, vt as a tiny fp32 tensor
    dcols = n_bat * DEC
    h_cols = enc_cols + dcols + 2 * H
    blob_in = nc.declare_dram_parameter("blob", [H, h_cols], fp16,
                                        isOutput=False)
    encT_in = blob_in[:, 0:enc_cols]
    decT_in = blob_in[:, enc_cols:enc_cols + dcols]
    w1_in = blob_in[:, enc_cols + dcols:enc_cols + dcols + H]
    w2_in = blob_in[:, enc_cols + dcols + H:]
    mask_in = nc.declare_dram_parameter("maskh", [DEC, enc_cols], fp16,
                                        isOutput=False)
    vt_in = nc.declare_dram_parameter("vt", [128, KCH], fp32, isOutput=False)
    # packed output: per block [masked | raw], fp16
    out_o = nc.declare_dram_parameter("out", [DEC, 2 * enc_cols], fp16,
                                      isOutput=True)

    # power-ladder closure (every power >= 2 built from lo*hi halves)
    allp = sorted(set(pows) | {1})
    changed = True
    while changed:
        changed = False
        for p in list(allp):
            if p > 1:
                for r in (p // 2, p - p // 2):
                    if r not in allp:
                        allp.append(r)
                        changed = True
        allp = sorted(set(allp))
    pows_all = [p for p in allp if p >= 2]
    dec_qs = sorted(set(q for _p, q, _c in terms_sorted))

    with tile.TileContext(nc) as tc:
        with (
            tc.tile_pool(name="weights", bufs=1) as wpool,
            tc.tile_pool(name="data", bufs=1) as dpool,
            tc.tile_pool(name="feat", bufs=1) as fpool,
            tc.tile_pool(name="fdecs", bufs=16) as spool,
            tc.tile_pool(name="io", bufs=2) as iopool,
            tc.tile_pool(name="ps_enc", bufs=2, space="PSUM") as pse,
            tc.tile_pool(name="ps_dec", bufs=1, space="PSUM") as psd,
            tc.tile_pool(name="ps_score", bufs=1, space="PSUM") as pss,
        ):
            vt = dpool.tile([128, KCH], fp32)
            nc.sync.dma_start(out=vt[:], in_=vt_in[:])

            w1 = []
            w2 = []
            decT = []
            for hc in range(HCH):
                t = wpool.tile([128, H], fp16, tag=f"w1_{hc}", name=f"w1_{hc}")
                nc.sync.dma_start(out=t[:], in_=w1_in[hc * 128:(hc + 1) * 128, :])
                w1.append(t)
                t = wpool.tile([128, H], fp16, tag=f"w2_{hc}", name=f"w2_{hc}")
                nc.sync.dma_start(out=t[:], in_=w2_in[hc * 128:(hc + 1) * 128, :])
                w2.append(t)
                t = dpool.tile([128, n_bat * DEC], fp16, tag=f"decT_{hc}",
                               name=f"decT_{hc}")
                nc.sync.dma_start(out=t[:], in_=decT_in[hc * 128:(hc + 1) * 128, :])
                decT.append(t)

            # vt broadcast to (128, KCH*DEC): col block kc is vt[:, kc]
            vt_wide = dpool.tile([128, KCH * DEC], fp16)
            ones = dpool.tile([128, DEC], fp32)
            nc.vector.memset(ones[:], 1.0)
            for kc in range(KCH):
                nc.scalar.activation(vt_wide[:, kc * DEC:(kc + 1) * DEC],
                                     ones[:], AF.Copy, scale=vt[:, kc:kc + 1])

            import concourse.bass as bass

            EWb = EW

            def emit_block(enc_col, dec_col, out_col):
                # ---- load this block's encoder cols ----
                encT = []
                for hc in range(HCH):
                    t = iopool.tile([128, EWb], fp16, tag=f"encT_{hc}",
                                    name=f"encT_{hc}")
                    nc.sync.dma_start(
                        out=t[:],
                        in_=encT_in[hc * 128:(hc + 1) * 128,
                                    bass.ds(enc_col, EWb)])
                    encT.append(t)
                mask_sb = iopool.tile([DEC, EWb], fp16, tag="mask")
                nc.sync.dma_start(out=mask_sb[:],
                                  in_=mask_in[:, bass.ds(enc_col, EWb)])

                # ---- stage 1: enc_t^T, dec_t^T (k on partitions, /tau) ----
                ps_dec = psd.tile([128, KCH * DEC], fp32, tag="psd")
                for kc in range(KCH):
                    for hc in range(HCH):
                        nc.tensor.matmul(
                            ps_dec[:, kc * DEC:(kc + 1) * DEC],
                            lhsT=w2[hc][:, kc * 128:(kc + 1) * 128],
                            rhs=decT[hc][:, bass.ds(dec_col, DEC)],
                            start=(hc == 0), stop=(hc == HCH - 1),
                        )

                za = {}
                zb = {}
                za[1] = fpool.tile([128, KCH * EWb], fp16, tag="za1", name="za1")
                zb[1] = fpool.tile([128, KCH * DEC], fp16, tag="zb1", name="zb1")
                HB = KCH * DEC // 2
                nc.scalar.activation(zb[1][:, :HB], ps_dec[:, :HB], AF.Tanh)
                nc.scalar.activation(zb[1][:, HB:], ps_dec[:, HB:], AF.Tanh)

                for kc in range(KCH):
                    ps_enc = pse.tile([128, EWb], fp32, tag="pse")
                    for hc in range(HCH):
                        nc.tensor.matmul(
                            ps_enc[:],
                            lhsT=w1[hc][:, kc * 128:(kc + 1) * 128],
                            rhs=encT[hc][:],
                            start=(hc == 0), stop=(hc == HCH - 1),
                        )
                    nc.scalar.activation(za[1][:, kc * EWb:(kc + 1) * EWb],
                                         ps_enc[:], AF.Tanh)

                if 0 in pows:
                    za[0] = fpool.tile([128, KCH * EWb], fp16, tag="za0",
                                       name="za0")
                    nc.vector.memset(za[0][:], 1.0)

                # ---- power ladders (DVE odd / ACT even) ----
                for p in pows_all:
                    lo, hi = p // 2, p - p // 2
                    te = fpool.tile([128, KCH * EWb], fp16, tag=f"za{p}", name=f"za{p}")
                    td = fpool.tile([128, KCH * DEC], fp16, tag=f"zb{p}", name=f"zb{p}")
                    if p % 2 == 0:
                        nc.scalar.activation(te[:], za[lo][:], AF.Square)
                        nc.scalar.activation(td[:], zb[lo][:], AF.Square)
                    else:
                        nc.vector.tensor_mul(te[:], za[lo][:], za[hi][:])
                        nc.vector.tensor_mul(td[:], zb[lo][:], zb[hi][:])
                    za[p] = te
                    zb[p] = td

                # ---- fold vt into dec atoms once: zb_v[q] = zb[q] * vt ----
                zb_v = {0: vt_wide}
                for q in dec_qs:
                    if q == 0:
                        continue
                    t = fpool.tile([128, KCH * DEC], fp16, tag=f"zbv{q}", name=f"zbv{q}")
                    nc.vector.tensor_mul(t[:], zb[q][:], vt_wide[:])
                    zb_v[q] = t

                # ---- group terms by za power p: gdec[p] = sum_q c*zbv[q],
                # then one matmul set per distinct p ----
                by_p = {}
                for p, q, cc in terms_sorted:
                    by_p.setdefault(p, []).append((q, float(cc)))
                p_list = sorted(by_p)

                ps_score = pss.tile([DEC, EWb], fp32, tag="pscore")
                n_mm = 0
                total_mm = len(p_list) * KCH
                for p in p_list:
                    qcs = by_p[p]
                    g = spool.tile([128, KCH * DEC], fp16, tag="fdecs")
                    nc.vector.tensor_scalar_mul(g[:], zb_v[qcs[0][0]][:],
                                                qcs[0][1])
                    for q, cc in qcs[1:]:
                        t2 = spool.tile([128, KCH * DEC], fp16, tag="fdect")
                        nc.vector.tensor_scalar_mul(t2[:], zb_v[q][:], cc)
                        g2 = spool.tile([128, KCH * DEC], fp16, tag="fdecs")
                        nc.vector.tensor_add(g2[:], g[:], t2[:])
                        g = g2
                    for kc in range(KCH):
                        nc.tensor.matmul(
                            ps_score[:],
                            lhsT=g[:, kc * DEC:(kc + 1) * DEC],
                            rhs=za[p][:, kc * EWb:(kc + 1) * EWb],
                            start=(n_mm == 0), stop=(n_mm == total_mm - 1),
                        )
                        n_mm += 1

                # ---- epilogue: [masked | raw] fp16, one DMA ----
                out_sb = iopool.tile([DEC, 2 * EWb], fp16, tag="out_sb")
                nc.vector.tensor_add(out_sb[:, :EWb], ps_score[:], mask_sb[:])
                nc.vector.tensor_copy(out_sb[:, EWb:], ps_score[:])
                nc.sync.dma_start(
                    out=out_o[:, bass.ds(out_col, 2 * EWb)],
                    in_=out_sb[:])

            if n_blk == 1:
                emit_block(0, 0, 0)
            else:
                # hardware loop: 4x fewer instructions in the module, which
                # cuts per-call BIR serialize/lower/cache-load host time
                with tc.For_i(0, n_blk, 1) as iv:
                    emit_block(iv * EWb, iv * DEC, iv * (2 * EWb))

    nc.finalize()
    return nc


def _prep_weights(W1, W2, vt):
    W1 = np.asarray(W1, dtype=np.float32)
    W2 = np.asarray(W2, dtype=np.float32)
    vt = np.asarray(vt, dtype=np.float32)
    w1h = (W1 / TAU).astype(np.float16)
    w2h = (W2 / TAU).astype(np.float16)
    vt_t = np.ascontiguousarray(vt.reshape(KCH, 128).T).astype(np.float32)
    return w1h, w2h, vt_t


def _get_nc(ncores=None):
    ncores = NCORES if ncores is None else ncores
    if ncores not in _COMPILED:
        _COMPILED[ncores] = _build_nc(ncores)
    return _COMPILED[ncores]


def prep_in_maps(decoder_state, encoder_outputs, mask, W1, W2, vt,
                 ncores=None):
    ncores = NCORES if ncores is None else ncores
    decoder_state = np.asarray(decoder_state, dtype=np.float32)
    encoder_outputs = np.asarray(encoder_outputs, dtype=np.float32)
    mask = np.asarray(mask, dtype=np.float32)
    w1h, w2h, vt_t = _prep_weights(W1, W2, vt)

    in_maps = []
    for blocks in _core_blocks(ncores):
        batches = sorted(set(b for b, _, _ in blocks))
        encT = np.concatenate(
            [encoder_outputs[b, off:off + w, :].T for b, off, w in blocks],
            axis=1)
        decT = np.concatenate(
            [decoder_state[b].T for b in batches], axis=1)
        msk = np.concatenate(
            [mask[b, :, off:off + w] for b, off, w in blocks], axis=1)
        blob = np.concatenate(
            [encT.astype(np.float16), decT.astype(np.float16), w1h, w2h],
            axis=1)
        in_maps.append({
            "blob": np.ascontiguousarray(blob),
            "maskh": np.ascontiguousarray(msk).astype(np.float16),
            "vt": vt_t,
        })
    return in_maps


def kernel(decoder_state, encoder_outputs, mask, W1, W2, vt):
    from concourse.bass_utils import run_bass_kernel_spmd

    nc = _get_nc()
    in_maps = prep_in_maps(decoder_state, encoder_outputs, mask, W1, W2, vt)
    _COMPILED["last_in_maps"] = in_maps
    res = run_bass_kernel_spmd(nc, in_maps, list(range(NCORES))).results

    log_score_masked = np.empty((B, DEC, ENC), dtype=np.float32)
    log_score = np.empty((B, DEC, ENC), dtype=np.float32)
    for core, blocks in enumerate(_core_blocks(NCORES)):
        out = res[core]["out"].astype(np.float32)
        col = 0
        for b, off, w in blocks:
            log_score_masked[b, :, off:off + w] = out[:, 2 * col:2 * col + w]
            log_score[b, :, off:off + w] = out[:, 2 * col + w:2 * col + 2 * w]
            col += w
    return (log_score_masked, log_score)
